# revision 45
# baseline (speedup 1.0000x reference)
"""BiDAF attention-flow kernel for Trainium2 (Bass/Tile), SPMD over 8 cores.

Math (per batch element b, one NeuronCore each):
    s[c,j]  = sc[c] + sq[j] + cq[c,j]            (scalar biases cancel)
    a       = softmax_j(s)
    c2q     = a @ e1                              (C,H)
    b_att   = softmax_c(max_j s)
    q2c     = b_att @ e2                          (H,)
    out     = [e2, c2q, e2*c2q, e2*q2c] @ w_red.T + b_red

Key tricks vs the obvious mapping:
  * sc[c] = sum_h wc[h]*e2t[h,c] is folded into the phase-A weights:
    e1w[h,j] = wcq[h]*e1t[h,j] + wc[h], so s (incl. sc) comes out of the
    same matmuls and max_j exp(s) IS the b_att numerator E — no separate
    exp(sc) pass, no broadcast multiplies.
  * the middle blocks of the reduction layer (c2q @ w2 + (e2*c2q) @ w3,
    ~20% of the output norm each) run in fp8-e4m3 DoubleRow matmuls:
    two k-tiles per matmul at 2x fp8 rate = 4x the bf16 row rate.
    c2q is normalized and scaled by 16 at the phase-B PSUM eviction
    (bcr = 16/L via a 1/16-matrix ones-matmul + reciprocal), weights are
    scaled by 256 on the host, and the 1/4096 descale rides the pass-1
    eviction. Measured end-to-end L2 error 1.1e-2 vs the 2e-2 budget;
    the dominant e2 block (95% of the norm) stays bf16.
  * E is broadcast on all partitions after the gpsimd max-allreduce, so
    S = sum_c E comes from a Scalar-engine activation accumulator and
    1/S from a tiny Vector reciprocal — nothing blocks the PE queue.
  * the b_att reduce (gpsimd) and the q2c mul-reduce accumulation run in
    two C-halves, with the first half launched mid-phase-A, so q2c and
    the folded pass-2 weights are ready long before pass 2 needs them.
  * reduction layer: pass1 = fp8 blocks 2+3, pass2 = bf16 e2-block with
    wsum = wrt[0:6] + q2c-folded wrt[6:12] (+bias), summed on eviction.
  * Phase B is ch-outer so c2q chunks complete early; phase A runs as
    two half-C passes in e2t stream order so compute starts while the
    second half of e2t is still loading.

Host does sharding/layout only: batch split, transposes, bf16/fp8 casts.
"""

import sys

import numpy as np
import ml_dtypes

if "/opt/trn_rl_repo" not in sys.path:
    sys.path.insert(0, "/opt/trn_rl_repo")

B, Q, C, H, OUT = 8, 512, 2048, 768, 300
HT, JT, CT = H // 128, Q // 128, C // 128  # 6, 4, 16
NCH, CHW = 4, 512  # c chunks
CPT = 4  # c-tiles per chunk
CH2 = C // 2
NWARM = 54
SC_X = 16.0    # c2q / m3 fp8 scale
SW_X = 256.0   # w2/w3 fp8 scale

bf16 = ml_dtypes.bfloat16
f8e4 = ml_dtypes.float8_e4m3fn

_CACHE = {}


def _build_bass():
    import concourse.tile as tile
    from concourse import mybir, bass_isa, library_config, bacc

    f32 = mybir.dt.float32
    b16 = mybir.dt.bfloat16
    fp8 = mybir.dt.float8e4
    AF = mybir.ActivationFunctionType
    OP = mybir.AluOpType
    DR = mybir.MatmulPerfMode.DoubleRow

    nc = bacc.Bacc("TRN2", target_bir_lowering=False, debug=False)

    e1_d = nc.dram_tensor("e1", [Q, H], b16, kind="ExternalInput").ap()
    e1t_d = nc.dram_tensor("e1t", [H, Q], b16, kind="ExternalInput").ap()
    e2t_d = nc.dram_tensor("e2t", [H, C], b16, kind="ExternalInput").ap()
    wrt_d = nc.dram_tensor("wrt", [12 * 128, OUT], b16, kind="ExternalInput").ap()
    w23_d = nc.dram_tensor("w23", [128, 12, OUT], fp8, kind="ExternalInput").ap()
    wpk_d = nc.dram_tensor("wpk", [128, 3 * HT], f32, kind="ExternalInput").ap()
    bred_d = nc.dram_tensor("bred", [1, OUT], b16, kind="ExternalInput").ap()
    out_d = nc.dram_tensor("out", [C, OUT], f32, kind="ExternalOutput").ap()

    with tile.TileContext(nc) as tc:
        with (
            tc.tile_pool(name="singles", bufs=1) as singles,
            tc.tile_pool(name="amo", bufs=2) as amop,
            tc.tile_pool(name="odma", bufs=4) as odp,
            tc.tile_pool(name="ps_mm", bufs=6, space="PSUM") as ps_mm,
            tc.tile_pool(name="ps_out", bufs=2, space="PSUM") as ps_out,
        ):
            # gpsimd: need the 'attn' ucode library for partition_all_reduce
            nc.gpsimd.load_library(library_config.attn)

            # ---- persistent SBUF tensors -------------------------------
            e1_sb = singles.tile([128, JT, H], b16)      # emb1, j on parts
            e1t_sb = singles.tile([128, HT, Q], b16)     # emb1.T, h on parts
            e1w_sb = singles.tile([128, HT, Q], b16)     # wcq*e1T + wc
            e2t_sb = singles.tile([128, HT, C], b16)     # emb2.T, h on parts
            wrt_sb = singles.tile([128, 12, OUT], b16)   # w_red.T blocks 1,4
            w23_sb = singles.tile([128, 12, OUT], fp8)   # fp8 w2/w3 pairs
            wq4_sb = singles.tile([128, HT, OUT], b16)   # q2c-folded tail
            wsum_sb = singles.tile([128, HT, OUT], b16)  # wrT[0:6] + wq4T
            wpk_sb = singles.tile([128, 3 * HT], f32)
            wq_sb = singles.tile([128, HT], b16)
            bred_sb = singles.tile([1, OUT], b16)
            ones_mat = singles.tile([128, 128], b16)
            c16_mat = singles.tile([128, 128], b16)      # 1/16 matrix for L
            ones_row_b = singles.tile([1, 128], b16)
            sq_sb = singles.tile([128, JT], f32)         # sq as columns
            pt_sb = singles.tile([128, JT, NCH, CHW], b16)  # P_T = exp(s+sq)
            c2q8_sb = singles.tile([128, 3, 2, C], fp8)  # 16*c2q, ht-major
            m38_sb = singles.tile([128, 3, 2, C], fp8)   # 16*e2*c2q
            macc = singles.tile([128, C], b16)           # col-max of P_T
            mall = singles.tile([128, C], b16)           # = E after all-reduce
            bcr_sb = singles.tile([128, C], f32)         # 16/L bcast
            bcrb_sb = singles.tile([128, C], b16)        # bf16 copy of bcr
            scol_sb = singles.tile([128, 1], f32)        # S = sum_c E
            rs_col = singles.tile([128, 1], f32)         # 1/S
            uh_sb = singles.tile([128, HT, 2], f32)      # q2c halves
            q2c_sb = singles.tile([128, HT], f32)
            out_sb = singles.tile([128, CT, OUT], f32)   # pass-1 partials

            # ---- loads in phase-A consumption order --------------------
            nc.sync.dma_start(
                out=e1t_sb, in_=e1t_d.rearrange("(t p) j -> p t j", p=128)
            )
            nc.sync.dma_start(out=wpk_sb, in_=wpk_d)
            wcq_sb = wpk_sb[:, 0:HT]
            nc.vector.memset(ones_mat, 1.0)
            nc.vector.memset(c16_mat, 1.0 / SC_X)
            nc.vector.memset(ones_row_b, 1.0)
            nc.vector.memset(macc, 0.0)
            nc.vector.tensor_copy(wq_sb, wpk_sb[:, 2 * HT : 3 * HT])

            # HAM warm-up: keep the PE busy while inputs stream in, so the
            # clock gate is at 8/8 when the real matmuls start
            wps = ps_mm.tile([128, CHW], f32, tag="mm", name="warm")
            for _ in range(NWARM):
                nc.tensor.matmul(wps[:, 0:128], ones_mat, ones_mat,
                                 start=True, stop=True)
            nc.vector.tensor_copy(rs_col, wps[:, 0:1])

            # e1w = wcq (per h) * e1T + wc  (sc rides along in the matmuls)
            e2t_r = e2t_d.rearrange("(t p) c -> p t c", p=128)
            for ht in range(HT):
                nc.vector.tensor_scalar(
                    e1w_sb[:, ht, :], e1t_sb[:, ht, :],
                    wcq_sb[:, ht : ht + 1],
                    wpk_sb[:, HT + ht : HT + ht + 1],
                    OP.mult, OP.add,
                )
            for hf in range(2):
                fsl = slice(hf * 2 * CHW, (hf + 1) * 2 * CHW)
                for ht in range(HT):
                    nc.sync.dma_start(
                        out=e2t_sb[:, ht, fsl], in_=e2t_r[:, ht, fsl]
                    )
            nc.sync.dma_start(
                out=e1_sb, in_=e1_d.rearrange("(t p) h -> p t h", p=128)
            )
            nc.sync.dma_start(
                out=wrt_sb, in_=wrt_d.rearrange("(t p) o -> p t o", p=128)
            )
            nc.sync.dma_start(out=w23_sb, in_=w23_d)
            nc.sync.dma_start(out=bred_sb, in_=bred_d)

            # ---- sq columns (tiny, feeds exp bias) ---------------------
            for jt in range(JT):
                ps = ps_mm.tile([128, CHW], f32, tag="mm")
                for ht in range(HT):
                    nc.tensor.matmul(
                        ps[:, 0:1],
                        e1t_sb[:, ht, jt * 128 : (jt + 1) * 128],
                        wq_sb[:, ht : ht + 1],
                        start=(ht == 0),
                        stop=(ht == HT - 1),
                    )
                nc.vector.tensor_copy(sq_sb[:, jt : jt + 1], ps[:, 0:1])

            # ---- phase A: sT matmuls, exp, running max -----------------
            # two half-C passes (first starts while e2t's second half
            # streams); jt outer / ht mid / ch inner within a pass.
            def emit_half_amr(hf):
                # u[h] += sum_c e2t[h,c]*E[c] over this C-half, on the
                # POOL engine (idle otherwise) to keep the DVE free
                hsl = slice(hf * CH2, (hf + 1) * CH2)
                for ht in range(HT):
                    amo = amop.tile([128, CH2], b16, tag="amo",
                                    name=f"am{hf}_{ht}")
                    nc.vector.affine_mul_reduce(
                        out=amo,
                        accum_out=uh_sb[:, ht, hf : hf + 1],
                        in0=e2t_sb[:, ht, hsl],
                        in1=mall[:, hsl],
                        scale=1.0,
                        bias=0.0,
                    )

            for hf in range(2):
                msl = slice(hf * 2 * CHW, (hf + 1) * 2 * CHW)
                for jt in range(JT):
                    sps = [
                        ps_mm.tile([128, CHW], f32, tag="mm",
                                   name=f"sps{hf}_{jt}_{i}")
                        for i in range(2)
                    ]
                    for ht in range(HT):
                        for chh in range(2):
                            ch = hf * 2 + chh
                            nc.tensor.matmul(
                                sps[chh],
                                e1w_sb[:, ht, jt * 128 : (jt + 1) * 128],
                                e2t_sb[:, ht, ch * CHW : (ch + 1) * CHW],
                                start=(ht == 0),
                                stop=(ht == HT - 1),
                            )
                    for chh in range(2):
                        ch = hf * 2 + chh
                        nc.scalar.activation(
                            out=pt_sb[:, jt, ch, :], in_=sps[chh],
                            func=AF.Exp,
                            bias=sq_sb[:, jt : jt + 1], scale=1.0,
                        )
                    nc.vector.tensor_max(
                        macc[:, msl], macc[:, msl],
                        pt_sb[:, jt, hf * 2 : hf * 2 + 2, :],
                    )
                # b_att numerator for this half: E = allreduce-max(macc)
                # (sc is inside s, so max_j exp(s) IS the numerator); the
                # q2c accumulation over this half follows on the DVE.
                nc.gpsimd.partition_all_reduce(
                    mall[:, hf * CH2 : (hf + 1) * CH2],
                    macc[:, hf * CH2 : (hf + 1) * CH2],
                    channels=128, reduce_op=bass_isa.ReduceOp.max,
                )
                if hf == 0:
                    emit_half_amr(0)

            # ---- L (as 1/16-scaled ones-matmul) -> bcr = 16/L, then ----
            # normalize pt in place: pt *= 16/L (all-bf16, 2x DVE rate).
            # Phase-B PSUM then holds 16*c2q directly, so the fp8
            # eviction is a constant-scale cast on the idle Scalar engine.
            for ch in range(NCH):
                csl = slice(ch * CHW, (ch + 1) * CHW)
                lps = ps_mm.tile([128, CHW], f32, tag="mm", name=f"lps{ch}")
                for jt in range(JT):
                    nc.tensor.matmul(
                        lps, c16_mat, pt_sb[:, jt, ch, :],
                        start=(jt == 0), stop=(jt == JT - 1),
                    )
                nc.vector.reciprocal_approx_fast(out=bcr_sb[:, csl], in_=lps)
                nc.vector.tensor_copy(bcrb_sb[:, csl], bcr_sb[:, csl])
                for jt in range(JT):
                    nc.vector.tensor_mul(
                        pt_sb[:, jt, ch, :], pt_sb[:, jt, ch, :],
                        bcrb_sb[:, csl],
                    )

            # S = sum_c E on the Scalar engine (mall is broadcast, so the
            # free-axis accumulator gives S on every partition)
            sdump = amop.tile([128, C], b16, tag="amo", name="sdump")
            nc.scalar.activation(
                out=sdump, in_=mall, func=AF.Copy, bias=0.0, scale=1.0,
                accum_out=scol_sb,
            )

            # ---- phase B: c2q matmuls, ch-outer; evict = *16/L in fp8 --
            for ch in range(NCH):
                csl = slice(ch * CHW, (ch + 1) * CHW)
                for ht in range(HT):
                    cps = ps_mm.tile([128, CHW], f32, tag="mm",
                                     name=f"cps{ch}_{ht}")
                    for jt in range(JT):
                        nc.tensor.matmul(
                            cps,
                            e1_sb[:, jt, ht * 128 : (ht + 1) * 128],
                            pt_sb[:, jt, ch, :],
                            start=(jt == 0), stop=(jt == JT - 1),
                        )
                    nc.scalar.activation(
                        out=c2q8_sb[:, ht // 2, ht % 2, csl], in_=cps,
                        func=AF.Copy, bias=0.0, scale=1.0,
                    )
                # m3 = e2t * (16*c2q), one batched op per chunk: early
                # chunks on the slow-but-idle POOL engine, late chunks on
                # the DVE so the pass-1 tail isn't Pool-bound.
                m3eng = nc.gpsimd if ch < 2 else nc.vector
                m3eng.tensor_mul(
                    m38_sb[:, :, :, csl], e2t_sb[:, :, csl],
                    c2q8_sb[:, :, :, csl],
                )
                if ch == 1:
                    emit_half_amr(1)

            # ---- q2c finalize: q2c = U / S, fold into wrT tail ---------
            nc.vector.reciprocal_approx_fast(out=rs_col, in_=scol_sb)
            nc.vector.reduce_sum(
                out=q2c_sb, in_=uh_sb, axis=mybir.AxisListType.X
            )
            nc.vector.tensor_scalar_mul(q2c_sb, q2c_sb, rs_col)
            for ht in range(HT):
                nc.vector.tensor_scalar_mul(
                    wq4_sb[:, ht, :], wrt_sb[:, 6 + ht, :],
                    q2c_sb[:, ht : ht + 1],
                )
                nc.vector.tensor_add(
                    wsum_sb[:, ht, :], wq4_sb[:, ht, :], wrt_sb[:, ht, :]
                )

            # ---- pass 1: fp8 DoubleRow blocks 2+3, 1/4096 descale ------
            for ct in range(CT):
                tsl = slice(ct * 128, (ct + 1) * 128)
                ops = ps_out.tile([128, OUT], f32, tag="out", name=f"ops{ct}")
                for pr in range(3):
                    nc.tensor.matmul(
                        ops, c2q8_sb[:, pr, :, tsl],
                        w23_sb[:, 2 * pr : 2 * pr + 2, :],
                        start=(pr == 0), stop=False, perf_mode=DR,
                    )
                for pr in range(3):
                    nc.tensor.matmul(
                        ops, m38_sb[:, pr, :, tsl],
                        w23_sb[:, 6 + 2 * pr : 8 + 2 * pr, :],
                        start=False, stop=(pr == 2), perf_mode=DR,
                    )
                nc.scalar.activation(
                    out=out_sb[:, ct, :], in_=ops, func=AF.Copy,
                    bias=0.0, scale=1.0 / (SC_X * SW_X),
                )

            # ---- pass 2: e2 block with q2c-folded weights + bias -------
            for ct in range(CT):
                tsl = slice(ct * 128, (ct + 1) * 128)
                obs = ps_out.tile([128, OUT], f32, tag="out", name=f"obs{ct}")
                for ht in range(HT):
                    nc.tensor.matmul(
                        obs, e2t_sb[:, ht, tsl], wsum_sb[:, ht, :],
                        start=(ht == 0), stop=False,
                    )
                nc.tensor.matmul(
                    obs, ones_row_b, bred_sb, start=False, stop=True,
                )
                od = odp.tile([128, OUT], f32, tag="od", name=f"od{ct}")
                nc.vector.tensor_add(od, obs, out_sb[:, ct, :])
                nc.sync.dma_start(out=out_d[tsl, :], in_=od)

    nc.compile()
    return nc


def _get_nc():
    if "nc" not in _CACHE:
        _CACHE["nc"] = _build_bass()
    return _CACHE["nc"]


def _in_maps(emb1, emb2, w_c, b_c, w_q, b_q, w_cq, b_cq, w_red, b_red):
    # host-side sharding + layout only: batch split, transposes, casts
    emb1 = np.asarray(emb1, np.float32)
    emb2 = np.asarray(emb2, np.float32)
    wcq = np.asarray(w_cq, np.float32).reshape(HT, 128).T
    wc = np.asarray(w_c, np.float32).reshape(HT, 128).T
    wq = np.asarray(w_q, np.float32).reshape(HT, 128).T
    wpk = np.ascontiguousarray(np.concatenate([wcq, wc, wq], axis=1))
    wrt_full = np.ascontiguousarray(np.asarray(w_red, np.float32).T)  # (4H, OUT)
    # bf16 blocks: e2 (k 0:768) and q2c-fold (k 2304:3072)
    wrt = np.concatenate(
        [wrt_full[0:768], wrt_full[2304:3072]], axis=0
    ).astype(bf16)
    # fp8 pairs for blocks 2+3: w23[p, blk*6 + 2*pr + sub, o]
    #   = 256 * wrt_full[768*(1+blk) + (2*pr+sub)*128 + p, o]
    w23 = np.empty((128, 12, OUT), np.float32)
    for blk in range(2):
        base = 768 * (1 + blk)
        for kt in range(6):
            w23[:, blk * 6 + kt, :] = wrt_full[
                base + kt * 128 : base + (kt + 1) * 128
            ]
    w23 = (w23 * SW_X).astype(f8e4)
    bred = np.asarray(b_red, np.float32).reshape(1, OUT).astype(bf16)
    maps = []
    for b in range(B):
        maps.append(
            {
                "e1": emb1[b].astype(bf16),
                "e1t": np.ascontiguousarray(emb1[b].T).astype(bf16),
                "e2t": np.ascontiguousarray(emb2[b].T).astype(bf16),
                "wrt": wrt,
                "w23": w23,
                "wpk": wpk,
                "bred": bred,
            }
        )
    return maps


def run(inputs, trace=False):
    from concourse.bass_utils import run_bass_kernel_spmd

    nc = _get_nc()
    maps = _in_maps(**inputs)
    res = run_bass_kernel_spmd(nc, maps, list(range(B)), trace=trace)
    out = np.stack([res.results[b]["out"] for b in range(B)], axis=0)
    return out.astype(np.float32), res


def kernel(**inputs) -> np.ndarray:
    out, _ = run(inputs, trace=False)
    return out


# revision 47
# speedup vs baseline: 1.1241x; 1.1241x over previous
"""BiDAF attention-flow kernel for Trainium2 (Bass/Tile), SPMD over 8 cores.

Math (per batch element b, one NeuronCore each):
    s[c,j]  = sc[c] + sq[j] + cq[c,j]            (scalar biases cancel)
    a       = softmax_j(s)
    c2q     = a @ e1                              (C,H)
    b_att   = softmax_c(max_j s)
    q2c     = b_att @ e2                          (H,)
    out     = [e2, c2q, e2*c2q, e2*q2c] @ w_red.T + b_red

Key tricks vs the obvious mapping:
  * sc[c] = sum_h wc[h]*e2t[h,c] is folded into the phase-A weights:
    e1w[h,j] = wcq[h]*e1t[h,j] + wc[h], so s (incl. sc) comes out of the
    same matmuls and max_j exp(s) IS the b_att numerator E — no separate
    exp(sc) pass, no broadcast multiplies.
  * the middle blocks of the reduction layer (c2q @ w2 + (e2*c2q) @ w3,
    ~20% of the output norm each) run in fp8-e4m3 DoubleRow matmuls:
    two k-tiles per matmul at 2x fp8 rate = 4x the bf16 row rate.
    c2q is normalized and scaled by 16 at the phase-B PSUM eviction
    (bcr = 16/L via a 1/16-matrix ones-matmul + reciprocal), weights are
    scaled by 256 on the host, and the 1/4096 descale rides the pass-1
    eviction. Measured end-to-end L2 error 1.1e-2 vs the 2e-2 budget;
    the dominant e2 block (95% of the norm) stays bf16.
  * E is broadcast on all partitions after the gpsimd max-allreduce, so
    S = sum_c E comes from a Scalar-engine activation accumulator and
    1/S from a tiny Vector reciprocal — nothing blocks the PE queue.
  * the b_att reduce (gpsimd) and the q2c mul-reduce accumulation run in
    two C-halves, with the first half launched mid-phase-A, so q2c and
    the folded pass-2 weights are ready long before pass 2 needs them.
  * reduction layer: pass1 = fp8 blocks 2+3, pass2 = bf16 e2-block with
    wsum = wrt[0:6] + q2c-folded wrt[6:12] (+bias), summed on eviction.
  * Phase B is ch-outer so c2q chunks complete early; phase A runs as
    two half-C passes in e2t stream order so compute starts while the
    second half of e2t is still loading.

Host does sharding/layout only: batch split, transposes, bf16/fp8 casts.
"""

import sys

import numpy as np
import ml_dtypes

if "/opt/trn_rl_repo" not in sys.path:
    sys.path.insert(0, "/opt/trn_rl_repo")

B, Q, C, H, OUT = 8, 512, 2048, 768, 300
HT, JT, CT = H // 128, Q // 128, C // 128  # 6, 4, 16
NCH, CHW = 4, 512  # c chunks
CPT = 4  # c-tiles per chunk
CH2 = C // 2
NWARM = 54
SC_X = 16.0    # c2q / m3 fp8 scale
SW_X = 256.0   # w2/w3 fp8 scale

bf16 = ml_dtypes.bfloat16
f8e4 = ml_dtypes.float8_e4m3fn

_CACHE = {}


def _build_bass():
    import concourse.tile as tile
    from concourse import mybir, bass_isa, library_config, bacc

    f32 = mybir.dt.float32
    b16 = mybir.dt.bfloat16
    fp8 = mybir.dt.float8e4
    AF = mybir.ActivationFunctionType
    OP = mybir.AluOpType
    DR = mybir.MatmulPerfMode.DoubleRow

    nc = bacc.Bacc("TRN2", target_bir_lowering=False, debug=False)

    e1_d = nc.dram_tensor("e1", [Q, H], b16, kind="ExternalInput").ap()
    e1t_d = nc.dram_tensor("e1t", [H, Q], b16, kind="ExternalInput").ap()
    e2t_d = nc.dram_tensor("e2t", [H, C], b16, kind="ExternalInput").ap()
    wrt_d = nc.dram_tensor("wrt", [12 * 128, OUT], b16, kind="ExternalInput").ap()
    w23_d = nc.dram_tensor("w23", [128, 12, OUT], fp8, kind="ExternalInput").ap()
    wpk_d = nc.dram_tensor("wpk", [128, 3 * HT], f32, kind="ExternalInput").ap()
    bred_d = nc.dram_tensor("bred", [1, OUT], b16, kind="ExternalInput").ap()
    out_d = nc.dram_tensor("out", [C, OUT], f32, kind="ExternalOutput").ap()

    with tile.TileContext(nc) as tc:
        with (
            tc.tile_pool(name="singles", bufs=1) as singles,
            tc.tile_pool(name="amo", bufs=2) as amop,
            tc.tile_pool(name="odma", bufs=4) as odp,
            tc.tile_pool(name="ps_mm", bufs=6, space="PSUM") as ps_mm,
            tc.tile_pool(name="ps_out", bufs=2, space="PSUM") as ps_out,
        ):
            # gpsimd: need the 'attn' ucode library for partition_all_reduce
            nc.gpsimd.load_library(library_config.attn)

            # ---- persistent SBUF tensors -------------------------------
            e1_sb = singles.tile([128, JT, H], b16)      # emb1, j on parts
            e1t_sb = singles.tile([128, HT, Q], b16)     # emb1.T, h on parts
            e1w_sb = singles.tile([128, HT, Q], b16)     # wcq*e1T + wc
            e2t_sb = singles.tile([128, HT, C], b16)     # emb2.T, h on parts
            wrt_sb = singles.tile([128, 12, OUT], b16)   # w_red.T blocks 1,4
            w23_sb = singles.tile([128, 12, OUT], fp8)   # fp8 w2/w3 pairs
            wq4_sb = singles.tile([128, HT, OUT], b16)   # q2c-folded tail
            wsum_sb = singles.tile([128, HT, OUT], b16)  # wrT[0:6] + wq4T
            wpk_sb = singles.tile([128, 3 * HT], f32)
            wq_sb = singles.tile([128, HT], b16)
            bred_sb = singles.tile([1, OUT], b16)
            ones_mat = singles.tile([128, 128], b16)
            c16_mat = singles.tile([128, 128], b16)      # 1/16 matrix for L
            ones_row_b = singles.tile([1, 128], b16)
            sq_sb = singles.tile([128, JT], f32)         # sq as columns
            pt_sb = singles.tile([128, JT, NCH, CHW], b16)  # P_T = exp(s+sq)
            c2q8_sb = singles.tile([128, 3, 2, C], fp8)  # 16*c2q, ht-major
            m38_sb = singles.tile([128, 3, 2, C], fp8)   # 16*e2*c2q
            macc = singles.tile([128, C], b16)           # col-max of P_T
            mall = singles.tile([128, C], b16)           # = E after all-reduce
            bcr_sb = singles.tile([128, C], f32)         # 16/L bcast
            bcrb_sb = singles.tile([128, C], b16)        # bf16 copy of bcr
            scol_sb = singles.tile([128, 1], f32)        # S = sum_c E
            rs_col = singles.tile([128, 1], f32)         # 1/S
            uh_sb = singles.tile([128, HT, 2], f32)      # q2c halves
            q2c_sb = singles.tile([128, HT], f32)
            out_sb = singles.tile([128, CT, OUT], f32)   # pass-1 partials

            # ---- loads in phase-A consumption order --------------------
            nc.sync.dma_start(
                out=e1t_sb, in_=e1t_d.rearrange("(t p) j -> p t j", p=128)
            )
            nc.sync.dma_start(out=wpk_sb, in_=wpk_d)
            wcq_sb = wpk_sb[:, 0:HT]
            nc.vector.memset(ones_mat, 1.0)
            nc.vector.memset(c16_mat, 1.0 / SC_X)
            nc.vector.memset(ones_row_b, 1.0)
            nc.vector.memset(macc, 0.0)
            nc.vector.tensor_copy(wq_sb, wpk_sb[:, 2 * HT : 3 * HT])

            # HAM warm-up: keep the PE busy while inputs stream in, so the
            # clock gate is at 8/8 when the real matmuls start
            wps = ps_mm.tile([128, CHW], f32, tag="mm", name="warm")
            for _ in range(NWARM):
                nc.tensor.matmul(wps[:, 0:128], ones_mat, ones_mat,
                                 start=True, stop=True)
            nc.vector.tensor_copy(rs_col, wps[:, 0:1])

            # e1w = wcq (per h) * e1T + wc  (sc rides along in the matmuls)
            e2t_r = e2t_d.rearrange("(t p) c -> p t c", p=128)
            for ht in range(HT):
                nc.vector.tensor_scalar(
                    e1w_sb[:, ht, :], e1t_sb[:, ht, :],
                    wcq_sb[:, ht : ht + 1],
                    wpk_sb[:, HT + ht : HT + ht + 1],
                    OP.mult, OP.add,
                )
            for hf in range(2):
                fsl = slice(hf * 2 * CHW, (hf + 1) * 2 * CHW)
                for ht in range(HT):
                    nc.sync.dma_start(
                        out=e2t_sb[:, ht, fsl], in_=e2t_r[:, ht, fsl]
                    )
            nc.sync.dma_start(
                out=e1_sb, in_=e1_d.rearrange("(t p) h -> p t h", p=128)
            )
            nc.sync.dma_start(
                out=wrt_sb, in_=wrt_d.rearrange("(t p) o -> p t o", p=128)
            )
            nc.sync.dma_start(out=w23_sb, in_=w23_d)
            nc.sync.dma_start(out=bred_sb, in_=bred_d)

            # ---- sq columns (tiny, feeds exp bias) ---------------------
            for jt in range(JT):
                ps = ps_mm.tile([128, CHW], f32, tag="mm")
                for ht in range(HT):
                    nc.tensor.matmul(
                        ps[:, 0:1],
                        e1t_sb[:, ht, jt * 128 : (jt + 1) * 128],
                        wq_sb[:, ht : ht + 1],
                        start=(ht == 0),
                        stop=(ht == HT - 1),
                    )
                nc.vector.tensor_copy(sq_sb[:, jt : jt + 1], ps[:, 0:1])

            # ---- phase A: sT matmuls, exp, running max -----------------
            # two half-C passes (first starts while e2t's second half
            # streams); jt outer / ht mid / ch inner within a pass.
            def emit_half_amr(hf):
                # u[h] += sum_c e2t[h,c]*E[c] over this C-half, on the
                # POOL engine (idle otherwise) to keep the DVE free
                hsl = slice(hf * CH2, (hf + 1) * CH2)
                for ht in range(HT):
                    amo = amop.tile([128, CH2], b16, tag="amo",
                                    name=f"am{hf}_{ht}")
                    nc.vector.affine_mul_reduce(
                        out=amo,
                        accum_out=uh_sb[:, ht, hf : hf + 1],
                        in0=e2t_sb[:, ht, hsl],
                        in1=mall[:, hsl],
                        scale=1.0,
                        bias=0.0,
                    )

            for hf in range(2):
                msl = slice(hf * 2 * CHW, (hf + 1) * 2 * CHW)
                for jt in range(JT):
                    sps = [
                        ps_mm.tile([128, CHW], f32, tag="mm",
                                   name=f"sps{hf}_{jt}_{i}")
                        for i in range(2)
                    ]
                    for ht in range(HT):
                        for chh in range(2):
                            ch = hf * 2 + chh
                            nc.tensor.matmul(
                                sps[chh],
                                e1w_sb[:, ht, jt * 128 : (jt + 1) * 128],
                                e2t_sb[:, ht, ch * CHW : (ch + 1) * CHW],
                                start=(ht == 0),
                                stop=(ht == HT - 1),
                            )
                    for chh in range(2):
                        ch = hf * 2 + chh
                        nc.scalar.activation(
                            out=pt_sb[:, jt, ch, :], in_=sps[chh],
                            func=AF.Exp,
                            bias=sq_sb[:, jt : jt + 1], scale=1.0,
                        )
                    nc.vector.tensor_max(
                        macc[:, msl], macc[:, msl],
                        pt_sb[:, jt, hf * 2 : hf * 2 + 2, :],
                    )
                # b_att numerator for this half: E = allreduce-max(macc)
                # (sc is inside s, so max_j exp(s) IS the numerator); the
                # q2c accumulation over this half follows on the DVE.
                nc.gpsimd.partition_all_reduce(
                    mall[:, hf * CH2 : (hf + 1) * CH2],
                    macc[:, hf * CH2 : (hf + 1) * CH2],
                    channels=128, reduce_op=bass_isa.ReduceOp.max,
                )
                if hf == 0:
                    emit_half_amr(0)

            # ---- L (as 1/16-scaled ones-matmul) -> bcr = 16/L, then ----
            # normalize pt in place: pt *= 16/L (all-bf16, 2x DVE rate).
            # Phase-B PSUM then holds 16*c2q directly, so the fp8
            # eviction is a constant-scale cast on the idle Scalar engine.
            for ch in range(NCH):
                csl = slice(ch * CHW, (ch + 1) * CHW)
                lps = ps_mm.tile([128, CHW], f32, tag="mm", name=f"lps{ch}")
                for jt in range(JT):
                    nc.tensor.matmul(
                        lps, c16_mat, pt_sb[:, jt, ch, :],
                        start=(jt == 0), stop=(jt == JT - 1),
                    )
                nc.vector.reciprocal_approx_fast(out=bcr_sb[:, csl], in_=lps)

            # S = sum_c E on the Scalar engine (mall is broadcast, so the
            # free-axis accumulator gives S on every partition)
            sdump = amop.tile([128, C], b16, tag="amo", name="sdump")
            nc.scalar.activation(
                out=sdump, in_=mall, func=AF.Copy, bias=0.0, scale=1.0,
                accum_out=scol_sb,
            )

            # ---- phase B: c2q matmuls, ch-outer; evict = *16/L in fp8 --
            for ch in range(NCH):
                csl = slice(ch * CHW, (ch + 1) * CHW)
                for ht in range(HT):
                    cps = ps_mm.tile([128, CHW], f32, tag="mm",
                                     name=f"cps{ch}_{ht}")
                    for jt in range(JT):
                        nc.tensor.matmul(
                            cps,
                            e1_sb[:, jt, ht * 128 : (ht + 1) * 128],
                            pt_sb[:, jt, ch, :],
                            start=(jt == 0), stop=(jt == JT - 1),
                        )
                    nc.vector.tensor_mul(
                        c2q8_sb[:, ht // 2, ht % 2, csl], cps, bcr_sb[:, csl]
                    )
                # m3 = e2t * (16*c2q), one batched op per chunk: early
                # chunks on the slow-but-idle POOL engine, the last on
                # the DVE so the pass-1 tail isn't Pool-bound.
                m3eng = nc.gpsimd if ch < 3 else nc.vector
                m3eng.tensor_mul(
                    m38_sb[:, :, :, csl], e2t_sb[:, :, csl],
                    c2q8_sb[:, :, :, csl],
                )
                if ch == 1:
                    emit_half_amr(1)

            # ---- q2c finalize: q2c = U / S, fold into wrT tail ---------
            nc.vector.reciprocal_approx_fast(out=rs_col, in_=scol_sb)
            nc.vector.reduce_sum(
                out=q2c_sb, in_=uh_sb, axis=mybir.AxisListType.X
            )
            nc.vector.tensor_scalar_mul(q2c_sb, q2c_sb, rs_col)
            for ht in range(HT):
                nc.vector.tensor_scalar_mul(
                    wq4_sb[:, ht, :], wrt_sb[:, 6 + ht, :],
                    q2c_sb[:, ht : ht + 1],
                )
                nc.vector.tensor_add(
                    wsum_sb[:, ht, :], wq4_sb[:, ht, :], wrt_sb[:, ht, :]
                )

            # ---- pass 1: fp8 DoubleRow blocks 2+3, 1/4096 descale ------
            for ct in range(CT):
                tsl = slice(ct * 128, (ct + 1) * 128)
                ops = ps_out.tile([128, OUT], f32, tag="out", name=f"ops{ct}")
                for pr in range(3):
                    nc.tensor.matmul(
                        ops, c2q8_sb[:, pr, :, tsl],
                        w23_sb[:, 2 * pr : 2 * pr + 2, :],
                        start=(pr == 0), stop=False, perf_mode=DR,
                    )
                for pr in range(3):
                    nc.tensor.matmul(
                        ops, m38_sb[:, pr, :, tsl],
                        w23_sb[:, 6 + 2 * pr : 8 + 2 * pr, :],
                        start=False, stop=(pr == 2), perf_mode=DR,
                    )
                nc.scalar.activation(
                    out=out_sb[:, ct, :], in_=ops, func=AF.Copy,
                    bias=0.0, scale=1.0 / (SC_X * SW_X),
                )

            # ---- pass 2: e2 block with q2c-folded weights + bias -------
            for ct in range(CT):
                tsl = slice(ct * 128, (ct + 1) * 128)
                obs = ps_out.tile([128, OUT], f32, tag="out", name=f"obs{ct}")
                for ht in range(HT):
                    nc.tensor.matmul(
                        obs, e2t_sb[:, ht, tsl], wsum_sb[:, ht, :],
                        start=(ht == 0), stop=False,
                    )
                nc.tensor.matmul(
                    obs, ones_row_b, bred_sb, start=False, stop=True,
                )
                od = odp.tile([128, OUT], f32, tag="od", name=f"od{ct}")
                nc.vector.tensor_add(od, obs, out_sb[:, ct, :])
                nc.sync.dma_start(out=out_d[tsl, :], in_=od)

    nc.compile()
    return nc


def _get_nc():
    if "nc" not in _CACHE:
        _CACHE["nc"] = _build_bass()
    return _CACHE["nc"]


def _in_maps(emb1, emb2, w_c, b_c, w_q, b_q, w_cq, b_cq, w_red, b_red):
    # host-side sharding + layout only: batch split, transposes, casts
    emb1 = np.asarray(emb1, np.float32)
    emb2 = np.asarray(emb2, np.float32)
    wcq = np.asarray(w_cq, np.float32).reshape(HT, 128).T
    wc = np.asarray(w_c, np.float32).reshape(HT, 128).T
    wq = np.asarray(w_q, np.float32).reshape(HT, 128).T
    wpk = np.ascontiguousarray(np.concatenate([wcq, wc, wq], axis=1))
    wrt_full = np.ascontiguousarray(np.asarray(w_red, np.float32).T)  # (4H, OUT)
    # bf16 blocks: e2 (k 0:768) and q2c-fold (k 2304:3072)
    wrt = np.concatenate(
        [wrt_full[0:768], wrt_full[2304:3072]], axis=0
    ).astype(bf16)
    # fp8 pairs for blocks 2+3: w23[p, blk*6 + 2*pr + sub, o]
    #   = 256 * wrt_full[768*(1+blk) + (2*pr+sub)*128 + p, o]
    w23 = np.empty((128, 12, OUT), np.float32)
    for blk in range(2):
        base = 768 * (1 + blk)
        for kt in range(6):
            w23[:, blk * 6 + kt, :] = wrt_full[
                base + kt * 128 : base + (kt + 1) * 128
            ]
    w23 = (w23 * SW_X).astype(f8e4)
    bred = np.asarray(b_red, np.float32).reshape(1, OUT).astype(bf16)
    maps = []
    for b in range(B):
        maps.append(
            {
                "e1": emb1[b].astype(bf16),
                "e1t": np.ascontiguousarray(emb1[b].T).astype(bf16),
                "e2t": np.ascontiguousarray(emb2[b].T).astype(bf16),
                "wrt": wrt,
                "w23": w23,
                "wpk": wpk,
                "bred": bred,
            }
        )
    return maps


def run(inputs, trace=False):
    from concourse.bass_utils import run_bass_kernel_spmd

    nc = _get_nc()
    maps = _in_maps(**inputs)
    res = run_bass_kernel_spmd(nc, maps, list(range(B)), trace=trace)
    out = np.stack([res.results[b]["out"] for b in range(B)], axis=0)
    return out.astype(np.float32), res


def kernel(**inputs) -> np.ndarray:
    out, _ = run(inputs, trace=False)
    return out
